# revision 1
# baseline (speedup 1.0000x reference)
"""AttnBlock (GroupNorm + 1x1-conv QKV + single-head spatial attention + proj
+ residual) on 8 Trainium2 NeuronCores.

Sharding: pure data-parallel over batch — 16 samples / 8 cores = 2 samples per
core; weights broadcast. No collectives; gather on host.

Key optimizations over the fp32r baseline:
  - proj folded into the v weights on host (W_pv = proj_w @ W_v), removing the
    proj GEMM entirely; its bias rides on the vT tiles (attention weights sum
    to 1, so (pv + b)·attn = pv·attn + b).
  - norm_w/norm_b folded into the QKV weights on host, so GroupNorm reduces to
    the pure affine (x - mu) * rstd per group.
  - k-bias dropped (it shifts logits by a per-query constant — softmax
    invariant).
  - all matmul operands in bf16 (1 cycle/row on the PE like fp32r — and
    measurably faster than IEEE fp16 on hardware — at half the SBUF/DMA
    traffic); PSUM accumulation stays fp32.
  - softmax denominators via a 3-level bf16 DVE add-tree over the 8 e j-tiles
    plus one ones-matmul per half (the 16 P=1 ones-matmuls ran ~3x slower
    than modeled).
  - x/weight/y DMAs ride only the two fast HWDGE rings (sync + scalar.
    gpsimd dma_start is the slow software-DGE path: ~46 GB/s).
  - two-sample schedule so the PE queue never waits on ACT/DVE:
    gn0 qk0 gn1 pv0 scores0 qk1 S0 pv1 scores1 o0[:4] S1 o0[4:]||o1.

Per-core kernel formulation (per sample, C=512 channels, N=1024 spatial):
  xn  = (x - mu_g) * rstd_g           (stats via DVE bn_stats + PE indicator)
  q   = Wq' xn + cq ; k = Wk' xn      (C on partitions, spatial free)
  vT  = xn^T Wpv'^T + b_out           (spatial on partitions, C free)
  e   = exp(k^T q * C^-0.5)           (logits O(1); no max-subtraction needed)
  S   = ones^T tree(e)                (DVE add-tree + tiny PE reduction)
  y   = x + (vT^T e) * (1/S)          (1/S broadcast across partitions via
                                       DRAM bounce)
"""

import numpy as np
import ml_dtypes
_BF16 = ml_dtypes.bfloat16

import concourse.bass as bass
import concourse.tile as tile
from concourse import bacc, mybir
from concourse.bass_utils import run_bass_kernel_spmd

B, C, H, W = 16, 512, 32, 32
N = H * W              # 1024 spatial positions
G = 32                 # groups
GS = C // G            # 16 channels per group
NCORES = 8
SPC = B // NCORES      # samples per core
EPS = 1e-6
SCALE = float(C) ** -0.5
KT = C // 128          # 4 channel tiles of 128
NT = N // 128          # 8 spatial tiles of 128
NH = N // 512          # 2 free-dim halves of 512

F32 = mybir.dt.float32
F32R = mybir.dt.float32r
F16 = mybir.dt.bfloat16

_BUILD_CACHE = {}
LAST_RESULT = None  # BassKernelResults of the most recent run (for test harness)


def _build():
    nc = bacc.Bacc("TRN2", target_bir_lowering=False, debug=False)

    x_ext = nc.declare_dram_parameter("x", [SPC, C, N], F32, isOutput=False)
    wqkT_ext = nc.declare_dram_parameter("wqkT", [C, 2 * C], F16, isOutput=False)
    wpvT_ext = nc.declare_dram_parameter("wpvT", [C, C], F16, isOutput=False)
    qbc_ext = nc.declare_dram_parameter("qb_col", [128, KT], F32, isOutput=False)
    boutbc_ext = nc.declare_dram_parameter("bout_bc", [128, C], F16, isOutput=False)
    ind_ext = nc.declare_dram_parameter("ind16", [128, 8], F32R, isOutput=False)
    indT_ext = nc.declare_dram_parameter("ind16T", [8, 128], F32R, isOutput=False)
    ones_ext = nc.declare_dram_parameter("ones16", [128, 1], F16, isOutput=False)
    y_ext = nc.declare_dram_parameter("y", [SPC, C, N], F32, isOutput=True)

    sdram = nc.dram_tensor("rs_bounce", [SPC, N], F32)

    Identity = mybir.ActivationFunctionType.Identity
    Copy = mybir.ActivationFunctionType.Copy
    Exp = mybir.ActivationFunctionType.Exp
    Sqrt = mybir.ActivationFunctionType.Sqrt
    mult = mybir.AluOpType.mult
    add = mybir.AluOpType.add

    with tile.TileContext(nc) as tc:
        with (
            tc.tile_pool(name="wpool", bufs=1) as wpool,
            tc.tile_pool(name="cpool", bufs=1) as cpool,
            tc.tile_pool(name="xpool", bufs=2) as xpool,
            tc.tile_pool(name="hpool", bufs=1) as hpool,
            tc.tile_pool(name="qpool", bufs=1) as qpool,
            tc.tile_pool(name="kpool", bufs=1) as kpool,
            tc.tile_pool(name="vpool", bufs=1) as vpool,
            tc.tile_pool(name="epool", bufs=1) as epool,
            tc.tile_pool(name="opool", bufs=4) as opool,
            tc.tile_pool(name="gnpool", bufs=2) as gnpool,
            tc.tile_pool(name="spool", bufs=1) as spool,
            tc.tile_pool(name="ps", bufs=7, space="PSUM") as ps,
            tc.tile_pool(name="psg", bufs=1, space="PSUM") as psg,
        ):
            # ---- x sample 0 first (256KB chunks across queues) ----
            x_tiles = [
                xpool.tile([128, KT, N], F32, tag="x", name=f"x_sb{s}")
                for s in range(SPC)
            ]
            # sync + scalar are the fast HWDGE rings; gpsimd DMA is the slow
            # software-DGE path, used only for small/late-needed constants.
            for kt in range(KT):
                eng = nc.sync if kt % 2 == 0 else nc.scalar
                eng.dma_start(
                    out=x_tiles[0][:, kt, :],
                    in_=x_ext.ap()[0, kt * 128 : (kt + 1) * 128, :],
                )

            # ---- small constants ----
            ind_sb = cpool.tile([128, 8], F32R)
            nc.sync.dma_start(out=ind_sb, in_=ind_ext.ap())
            indT_sb = cpool.tile([8, 128], F32R)
            nc.sync.dma_start(out=indT_sb, in_=indT_ext.ap())
            qb_col = cpool.tile([128, KT], F32)
            nc.gpsimd.dma_start(out=qb_col, in_=qbc_ext.ap())
            bout_bc = cpool.tile([128, C], F16)
            nc.gpsimd.dma_start(out=bout_bc, in_=boutbc_ext.ap())
            ones_col = cpool.tile([128, 1], F16)
            nc.gpsimd.dma_start(out=ones_col, in_=ones_ext.ap())
            eps_sb = cpool.tile([8, 1], F32)
            nc.vector.memset(eps_sb, EPS)
            warm_sb = cpool.tile([8, 1], F32)
            nc.scalar.activation(out=warm_sb, in_=eps_sb, func=Sqrt)
            nc.scalar.activation(out=warm_sb, in_=eps_sb, func=Identity)

            # ---- weights (kt0 chunks first so qk can start early) ----
            wqk_sb = wpool.tile([128, KT, 2 * C], F16)
            wpv_sb = wpool.tile([128, KT, C], F16)
            for kt in range(KT):
                eng = nc.scalar if kt % 2 == 0 else nc.sync
                eng.dma_start(
                    out=x_tiles[1][:, kt, :],
                    in_=x_ext.ap()[1, kt * 128 : (kt + 1) * 128, :],
                )
            for kt in range(KT):
                nc.sync.dma_start(
                    out=wqk_sb[:, kt, :],
                    in_=wqkT_ext.ap()[kt * 128 : (kt + 1) * 128, :],
                )
            for kt in range(KT):
                nc.gpsimd.dma_start(
                    out=wpv_sb[:, kt, :],
                    in_=wpvT_ext.ap()[kt * 128 : (kt + 1) * 128, :],
                )

            h_tiles = [
                hpool.tile([128, KT, N], F16, tag=f"h{s}", name=f"h{s}")
                for s in range(SPC)
            ]

            def gn_stats(s):
                """GroupNorm stats for sample s -> mr (8,KT,2) F32R
                [-mean*rstd... bias,scale]. bn_stats per chunk (DVE, paced by
                x DMA); all small chain ops batched across kt as slabs."""
                x_sb = x_tiles[s]
                mv = gnpool.tile([128, KT, 2], F32, tag="mv", name=f"mv{s}")
                stat_tiles = []
                for kt in range(KT):
                    stats = gnpool.tile(
                        [128, 2, 6], F32, tag=f"stats{kt}", name=f"stats{s}_{kt}"
                    )
                    stat_tiles.append(stats)
                    for sg in range(2):
                        nc.vector.bn_stats(
                            out=stats[:, sg, :],
                            in_=x_sb[:, kt, sg * 512 : (sg + 1) * 512],
                        )
                for kt in range(KT):
                    nc.vector.bn_aggr(out=mv[:, kt, :], in_=stat_tiles[kt])
                # [E[x], E[x^2]] = [mean, var + mean^2], one slab op each
                s2f = gnpool.tile([128, KT, 2], F32, tag="s2f", name=f"s2f{s}")
                nc.vector.tensor_mul(s2f[:, :, 1], mv[:, :, 0], mv[:, :, 0])
                nc.vector.tensor_add(s2f[:, :, 1], s2f[:, :, 1], mv[:, :, 1])
                nc.vector.tensor_copy(s2f[:, :, 0], mv[:, :, 0])
                s2 = gnpool.tile([128, KT, 2], F32R, tag="s2", name=f"s2_{s}")
                nc.vector.tensor_copy(s2, s2f)
                ps_gs = psg.tile([8, KT, 2], F32, tag="gn", name=f"ps_gs{s}")
                for kt in range(KT):
                    nc.tensor.matmul(
                        ps_gs[:, kt, :], ind_sb, s2[:, kt, :], start=True, stop=True
                    )
                # batched group mean / rstd chain across all kt
                gs = gnpool.tile([8, KT, 2], F32, tag="gs", name=f"gs{s}")
                nc.vector.tensor_scalar_mul(gs, ps_gs, 1.0 / GS)
                msq = gnpool.tile([8, KT], F32, tag="msq", name=f"msq{s}")
                nc.vector.tensor_mul(msq, gs[:, :, 0], gs[:, :, 0])
                nc.vector.tensor_sub(gs[:, :, 1], gs[:, :, 1], msq)
                nc.scalar.activation(
                    out=gs[:, :, 1], in_=gs[:, :, 1], func=Sqrt, bias=eps_sb
                )
                nc.vector.reciprocal(gs[:, :, 1], gs[:, :, 1])
                # fold bias in place: gs[...,0] = -mean * rstd
                nc.vector.tensor_mul(gs[:, :, 0], gs[:, :, 0], gs[:, :, 1])
                nc.vector.tensor_scalar_mul(gs[:, :, 0], gs[:, :, 0], -1.0)
                mr = gnpool.tile([8, KT, 2], F32R, tag="mr", name=f"mr{s}")
                nc.vector.tensor_copy(mr, gs)
                return mr

            def gn_apply(s, mr, engines):
                """Broadcast stats to channels (one matmul for all kt), apply
                (x-mu)*rstd -> h bf16."""
                x_sb = x_tiles[s]
                ps_bc = psg.tile([128, KT, 2], F32, tag="gn", name=f"ps_bc{s}")
                nc.tensor.matmul(ps_bc, indT_sb, mr, start=True, stop=True)
                scb = gnpool.tile([128, KT, 2], F32, tag="scb", name=f"scb{s}")
                nc.vector.tensor_copy(scb, ps_bc)
                for kt in range(KT):
                    if engines[kt] == "v":
                        nc.vector.tensor_scalar(
                            out=h_tiles[s][:, kt, :],
                            in0=x_sb[:, kt, :],
                            scalar1=scb[:, kt, 1:2],
                            scalar2=scb[:, kt, 0:1],
                            op0=mult,
                            op1=add,
                        )
                    else:
                        nc.scalar.activation(
                            out=h_tiles[s][:, kt, :], in_=x_sb[:, kt, :],
                            func=Identity, scale=scb[:, kt, 1:2],
                            bias=scb[:, kt, 0:1],
                        )

            def qk(s, q_sb, k_sb):
                """q,k = Wqk' @ h. k first per ih half (scores need k tiles
                of the matching jt range first); q bias via ACT, k plain copy
                split ACT/DVE."""
                h_sb = h_tiles[s]
                for ih in range(NH):
                    for ot in list(range(4, 8)) + list(range(4)):
                        pm = ps.tile([128, 512], F32, tag="mm")
                        for kt in range(KT):
                            nc.tensor.matmul(
                                pm,
                                wqk_sb[:, kt, ot * 128 : (ot + 1) * 128],
                                h_sb[:, kt, ih * 512 : (ih + 1) * 512],
                                start=(kt == 0),
                                stop=(kt == KT - 1),
                            )
                        if ot < 4:
                            nc.scalar.activation(
                                out=q_sb[:, ot, ih * 512 : (ih + 1) * 512],
                                in_=pm,
                                func=Identity,
                                bias=qb_col[:, ot : ot + 1],
                            )
                        else:
                            oc = ot - 4
                            dest = k_sb[:, oc, ih * 512 : (ih + 1) * 512]
                            if oc % 2 == 0:
                                nc.scalar.activation(out=dest, in_=pm, func=Copy)
                            else:
                                nc.vector.tensor_copy(dest, pm)
                return q_sb, k_sb

            def pv(s, vT_sb):
                """vT = h^T @ Wpv'^T + b_out (row-broadcast bias via DVE)."""
                h_sb = h_tiles[s]
                for nt in range(NT):
                    pm = ps.tile([128, 512], F32, tag="mm")
                    for kt in range(KT):
                        nc.tensor.matmul(
                            pm,
                            h_sb[:, kt, nt * 128 : (nt + 1) * 128],
                            wpv_sb[:, kt, :],
                            start=(kt == 0),
                            stop=(kt == KT - 1),
                        )
                    nc.vector.tensor_add(vT_sb[:, nt, :], pm, bout_bc)
                return vT_sb

            def scores(s, q_sb, k_sb, e_sb):
                # s = k^T q (keys on partitions); e = exp(s * scale) fp16
                for ih in range(NH):
                    for jt in range(NT):
                        pm = ps.tile([128, 512], F32, tag="mm")
                        for ct in range(KT):
                            nc.tensor.matmul(
                                pm,
                                k_sb[:, ct, jt * 128 : (jt + 1) * 128],
                                q_sb[:, ct, ih * 512 : (ih + 1) * 512],
                                start=(ct == 0),
                                stop=(ct == KT - 1),
                            )
                        nc.scalar.activation(
                            out=e_sb[:, jt, ih * 512 : (ih + 1) * 512],
                            in_=pm,
                            func=Exp,
                            scale=SCALE,
                        )
                return e_sb

            def s_tree(s, e_sb):
                """Reduce e over the 8 j-tiles with 3 wide bf16 DVE adds."""
                t4 = spool.tile([128, 4, N], F16, tag=f"t4_{s}", name=f"t4_{s}")
                t2 = spool.tile([128, 2, N], F16, tag=f"t2_{s}", name=f"t2_{s}")
                t1 = spool.tile([128, N], F16, tag=f"t1_{s}", name=f"t1_{s}")
                nc.vector.tensor_add(t4, e_sb[:, 0:4, :], e_sb[:, 4:8, :])
                nc.vector.tensor_add(t2, t4[:, 0:2, :], t4[:, 2:4, :])
                nc.vector.tensor_add(t1, t2[:, 0, :], t2[:, 1, :])
                return t1

            def s_denom(s, t1):
                """S = ones^T t1 (one matmul per half); 1/S broadcast via
                DRAM bounce."""
                recipS = spool.tile([1, N], F32, tag=f"recipS{s}", name=f"recipS{s}")
                for ih in range(NH):
                    pS = ps.tile([1, 512], F32, tag="mm", name=f"pS{s}_{ih}")
                    nc.tensor.matmul(
                        pS,
                        ones_col,
                        t1[:, ih * 512 : (ih + 1) * 512],
                        start=True,
                        stop=True,
                    )
                    nc.vector.reciprocal_approx_fast(
                        out=recipS[:, ih * 512 : (ih + 1) * 512], in_=pS
                    )
                nc.sync.dma_start(out=sdram.ap()[s].unsqueeze(0), in_=recipS)
                rSbc = spool.tile([128, N], F32, tag=f"rSbc{s}", name=f"rSbc{s}")
                nc.sync.dma_start(
                    out=rSbc, in_=sdram.ap()[s].partition_broadcast(128)
                )
                return rSbc

            def o_group(s, vT_sb, e_sb, rSbc, ct, ih, dma_eng, gp_add=False):
                """One output tile: o = vT^T e, normalize by 1/S, add residual
                in place into the (now dead) x tile, stream to DRAM."""
                x_sb = x_tiles[s]
                pm = ps.tile([128, 512], F32, tag="mm")
                for jt in range(NT):
                    nc.tensor.matmul(
                        pm,
                        vT_sb[:, jt, ct * 128 : (ct + 1) * 128],
                        e_sb[:, jt, ih * 512 : (ih + 1) * 512],
                        start=(jt == 0),
                        stop=(jt == NT - 1),
                    )
                t = opool.tile([128, 512], F32, tag="onorm")
                nc.vector.tensor_mul(t, pm, rSbc[:, ih * 512 : (ih + 1) * 512])
                aeng = nc.gpsimd if gp_add else nc.vector
                aeng.tensor_add(
                    x_sb[:, ct, ih * 512 : (ih + 1) * 512],
                    t,
                    x_sb[:, ct, ih * 512 : (ih + 1) * 512],
                )
                dma_eng.dma_start(
                    out=y_ext.ap()[
                        s, ct * 128 : (ct + 1) * 128, ih * 512 : (ih + 1) * 512
                    ],
                    in_=x_sb[:, ct, ih * 512 : (ih + 1) * 512],
                )

            # SBUF tiles for the attention intermediates
            q0 = qpool.tile([128, KT, N], F16, tag="q0", name="q0")
            k0 = kpool.tile([128, KT, N], F16, tag="k0", name="k0")
            q1 = qpool.tile([128, KT, N], F16, tag="q1", name="q1")
            k1 = kpool.tile([128, KT, N], F16, tag="k1", name="k1")
            vT0 = vpool.tile([128, NT, C], F16, tag="vT0", name="vT0")
            vT1 = vpool.tile([128, NT, C], F16, tag="vT1", name="vT1")
            e0 = epool.tile([128, NT, N], F16, tag="e0", name="e0")
            e1 = epool.tile([128, NT, N], F16, tag="e1", name="e1")

            # ---- interleaved two-sample schedule ----
            mr0 = gn_stats(0)
            gn_apply(0, mr0, "avav")
            nc.scalar.activation(out=warm_sb, in_=eps_sb, func=Copy)
            nc.scalar.activation(out=warm_sb, in_=eps_sb, func=Exp)
            qk(0, q0, k0)
            with tc.tile_wait_until(0.020):
                mr1 = gn_stats(1)       # floored: don't preempt chain0 on DVE
            gn_apply(1, mr1, "aaaa")    # PE mms land after qk0, gated ~chain1
            pv(0, vT0)
            scores(0, q0, k0, e0)
            t1_0 = s_tree(0, e0)
            qk(1, q1, k1)
            rS0 = s_denom(0, t1_0)      # exp0+tree0 done during qk1
            pv(1, vT1)                  # covers recip0 + DRAM bounce
            scores(1, q1, k1, e1)
            t1_1 = s_tree(1, e1)
            ogrps = [(ct, ih) for ct in range(KT) for ih in range(NH)]
            for gi in range(4):
                o_group(0, vT0, e0, rS0, *ogrps[gi],
                        nc.sync if gi % 2 == 0 else nc.scalar)
            rS1 = s_denom(1, t1_1)
            for gi in range(4, 8):
                o_group(0, vT0, e0, rS0, *ogrps[gi],
                        nc.sync if gi % 2 == 0 else nc.scalar)
                o_group(1, vT1, e1, rS1, *ogrps[gi - 4],
                        nc.scalar if gi % 2 == 0 else nc.sync,
                        gp_add=(gi % 2 == 1))
            for gi in range(4, 8):
                o_group(1, vT1, e1, rS1, *ogrps[gi],
                        nc.sync if gi % 2 == 0 else nc.scalar,
                        gp_add=(gi % 2 == 0))

    nc.compile()
    return nc


def _get_nc():
    if "nc" not in _BUILD_CACHE:
        _BUILD_CACHE["nc"] = _build()
    return _BUILD_CACHE["nc"]


def kernel(x, norm_w, norm_b, qkv_w, qkv_b, proj_w, proj_b, _trace=False):
    global LAST_RESULT
    nc = _get_nc()

    x = np.asarray(x, dtype=np.float32).reshape(B, C, N)
    norm_w = np.asarray(norm_w, dtype=np.float64)
    norm_b = np.asarray(norm_b, dtype=np.float64)
    qkv_w = np.asarray(qkv_w, dtype=np.float64)
    qkv_b = np.asarray(qkv_b, dtype=np.float64)
    proj_w = np.asarray(proj_w, dtype=np.float64)
    proj_b = np.asarray(proj_b, dtype=np.float64)

    # fold norm affine + proj into the weights (exact, in float64)
    Wq = qkv_w[:C] * norm_w[None, :]
    Wk = qkv_w[C : 2 * C] * norm_w[None, :]
    Wpv = proj_w @ (qkv_w[2 * C :] * norm_w[None, :])
    cq = qkv_w[:C] @ norm_b + qkv_b[:C]          # q bias (k bias dropped)
    b_out = proj_w @ (qkv_w[2 * C :] @ norm_b + qkv_b[2 * C :]) + proj_b

    wqkT = np.ascontiguousarray(
        np.concatenate([Wq, Wk], axis=0).T.astype(_BF16)
    )
    wpvT = np.ascontiguousarray(Wpv.T.astype(_BF16))
    qb_col = np.ascontiguousarray(
        cq.astype(np.float32).reshape(KT, 128).T
    )
    bout_bc = np.ascontiguousarray(
        np.broadcast_to(b_out.astype(_BF16), (128, C))
    )
    ind16 = np.zeros((128, 8), dtype=np.float32)
    for p in range(128):
        ind16[p, p // GS] = 1.0
    ind16T = np.ascontiguousarray(ind16.T)

    shared = {
        "wqkT": wqkT,
        "wpvT": wpvT,
        "qb_col": qb_col,
        "bout_bc": bout_bc,
        "ind16": ind16,
        "ind16T": ind16T,
        "ones16": np.ones((128, 1), dtype=_BF16),
    }
    in_maps = [
        {"x": np.ascontiguousarray(x[c * SPC : (c + 1) * SPC]), **shared}
        for c in range(NCORES)
    ]
    res = run_bass_kernel_spmd(nc, in_maps, list(range(NCORES)), trace=_trace)
    LAST_RESULT = res
    out = np.concatenate([res.results[i]["y"] for i in range(NCORES)], axis=0)
    return out.reshape(B, C, H, W)

